# revision 1
# baseline (speedup 1.0000x reference)
"""Trainium2 Bass kernel for nn_Denoiser_73598559584966.

Full-sequence self-attention (Q=K=V, no scaling) over x: [4, 16, 16, 16, 64]
  t = x.reshape(B, 4096, 64); out = softmax(t @ t^T) @ t

Sharding: 8 cores = 4 batches x 2 query-halves. Each core handles 2048
queries against the full 4096 keys/values of its batch. No collectives;
host shards inputs and gathers outputs.

Device algorithm per core (all matmuls fp32r = 11-bit-mantissa mode at
full PE rate; q/k are split hi/lo on host so scores are fp32-accurate):
  S^T[keys, q] = (k_hi|k_lo)^T(q_hi|q_hi) + (k_hi|1)^T(q_lo|-B)
               = q.k - B_i   (B_i = |q_i| max_j|k_j| >= rowmax, safe exp bound)
  P = exp(S^T)             (ScalarE, PSUM -> fp32r SBUF)
  O^T[65, q] = sum_kt (V_kt|1)^T P_kt   (row 64 = softmax denominator)
  out[q, c] = transpose(O^T) cols 0..63 * 1/col64  (PE transpose + DVE)
"""
import numpy as np

B_, D_, H_, W_, C_ = 4, 16, 16, 16, 64
NTOK = D_ * H_ * W_          # 4096 tokens per batch
NQ = NTOK // 2               # 2048 queries per core
NCORES = 8
NKT = NTOK // 128            # 32 key tiles
NCH = 2                      # query chunks per core
CHW = NQ // NCH              # 1024 queries per chunk
NJ = CHW // 512              # 512-wide matmul slices per chunk

_CACHE = {}


def _round11(x):
    """Round fp32 to 11 explicit mantissa bits (fp32r grid), RNE."""
    u = np.ascontiguousarray(x, np.float32).view(np.uint32)
    bias = ((u >> 12) & 1) + np.uint32((1 << 11) - 1)
    u = (u + bias) & np.uint32(0xFFFFF000)
    return u.view(np.float32)


def _build_nc():
    import concourse.bacc as bacc
    import concourse.mybir as mybir
    from concourse.tile import TileContext

    f32 = mybir.dt.float32
    f32r = mybir.dt.float32r
    nc = bacc.Bacc("TRN2", target_bir_lowering=False, debug=False)

    khl = nc.dram_tensor("khl", [128, NTOK], f32r, kind="ExternalInput")
    khi1 = nc.dram_tensor("khi1", [65, NTOK], f32r, kind="ExternalInput")
    qhh = nc.dram_tensor("qhh", [128, NQ], f32r, kind="ExternalInput")
    qlo1 = nc.dram_tensor("qlo1", [65, NQ], f32r, kind="ExternalInput")
    vpk = nc.dram_tensor("vpk", [128, NKT * 65], f32r, kind="ExternalInput")
    ident = nc.dram_tensor("ident", [65, 65], f32, kind="ExternalInput")
    out = nc.dram_tensor("out", [NQ, C_], f32, kind="ExternalOutput")
    out_r = out.rearrange("(a p) c -> p a c", p=128)  # a = chunk*8 + tile

    with TileContext(nc) as tc:
        with (
            tc.tile_pool(name="const", bufs=1) as const,
            tc.tile_pool(name="pp", bufs=3) as pp,
            tc.tile_pool(name="sbo", bufs=2) as sbo,
            tc.tile_pool(name="ps_s", bufs=2, space="PSUM") as ps_s,
            tc.tile_pool(name="ps_o", bufs=1, space="PSUM") as ps_o,
            tc.tile_pool(name="ps_t", bufs=2, space="PSUM") as ps_t,
        ):
            khl_t = const.tile([128, NTOK], f32r, tag="khl")
            nc.sync.dma_start(out=khl_t, in_=khl[:, :])
            khi1_t = const.tile([65, NTOK], f32r, tag="khi1")
            nc.sync.dma_start(out=khi1_t, in_=khi1[:, :])
            qhh_t = const.tile([128, NQ], f32r, tag="qhh")
            nc.sync.dma_start(out=qhh_t, in_=qhh[:, :])
            qlo1_t = const.tile([65, NQ], f32r, tag="qlo1")
            nc.sync.dma_start(out=qlo1_t, in_=qlo1[:, :])
            vpk_t = const.tile([128, NKT * 65], f32r, tag="vpk")
            nc.sync.dma_start(out=vpk_t, in_=vpk[:, :])
            ident_t = const.tile([65, 65], f32, tag="ident")
            nc.sync.dma_start(out=ident_t, in_=ident[:, :])

            for ch in range(NCH):
                q0 = ch * CHW
                o_acc = ps_o.tile([65, CHW], f32, tag="oacc")
                for kt in range(NKT):
                    s_t = ps_s.tile([128, CHW], f32, tag="s")
                    for j in range(NJ):
                        js = slice(j * 512, (j + 1) * 512)
                        qs = slice(q0 + j * 512, q0 + (j + 1) * 512)
                        nc.tensor.matmul(
                            s_t[:, js],
                            khl_t[:, kt * 128:(kt + 1) * 128],
                            qhh_t[:, qs],
                            start=True, stop=False,
                        )
                        nc.tensor.matmul(
                            s_t[:, js],
                            khi1_t[:, kt * 128:(kt + 1) * 128],
                            qlo1_t[:, qs],
                            start=False, stop=True,
                        )
                    import concourse.mybir as _mybir
                    p_t = pp.tile([128, CHW], f32r, tag="p")
                    nc.scalar.activation(
                        p_t, s_t, _mybir.ActivationFunctionType.Exp
                    )
                    for j in range(NJ):
                        js = slice(j * 512, (j + 1) * 512)
                        nc.tensor.matmul(
                            o_acc[:, js],
                            vpk_t[:, kt * 65:kt * 65 + 65],
                            p_t[:, js],
                            start=(kt == 0), stop=(kt == NKT - 1),
                            skip_group_check=True,
                        )
                o_sb = sbo.tile([65, CHW], f32, tag="osb")
                nc.vector.tensor_copy(o_sb, o_acc)
                out_sb = sbo.tile([128, 8 * C_], f32, tag="outsb")
                for tt in range(8):
                    tr = ps_t.tile([128, 65], f32, tag="tr")
                    nc.tensor.transpose(
                        tr, o_sb[:, tt * 128:(tt + 1) * 128], ident_t
                    )
                    inv = sbo.tile([128, 1], f32, tag="inv")
                    nc.vector.reciprocal(inv, tr[:, 64:65])
                    nc.vector.tensor_scalar_mul(
                        out_sb[:, tt * C_:(tt + 1) * C_], tr[:, 0:C_], inv
                    )
                nc.sync.dma_start(
                    out=out_r[:, ch * 8:(ch + 1) * 8, :],
                    in_=out_sb.rearrange("p (a c) -> p a c", c=C_),
                )
    nc.compile()
    return nc


def _prep_inputs(x):
    """Host-side shard + operand marshaling. Returns list of 8 in_maps."""
    t = np.ascontiguousarray(x, np.float32).reshape(B_, NTOK, C_)
    ident = np.eye(65, dtype=np.float32)
    in_maps = []
    for b in range(B_):
        kv = t[b]                                   # [4096, 64]
        k_hi = _round11(kv)
        k_lo = (kv - k_hi).astype(np.float32)
        kmax = float(np.linalg.norm(kv.astype(np.float64), axis=1).max())
        khl = np.concatenate([k_hi.T, k_lo.T]).astype(np.float32)
        khi1 = np.concatenate(
            [k_hi.T, np.ones((1, NTOK), np.float32)]).astype(np.float32)
        vpk = np.concatenate(
            [np.concatenate([kv[i * 128:(i + 1) * 128],
                             np.ones((128, 1), np.float32)], axis=1)
             for i in range(NKT)], axis=1).astype(np.float32)  # [128, 32*65]
        for h in range(2):
            q = t[b, h * NQ:(h + 1) * NQ]           # [2048, 64]
            q_hi = _round11(q)
            q_lo = (q - q_hi).astype(np.float32)
            qn = np.linalg.norm(q.astype(np.float64), axis=1)
            bias = (qn * kmax + 0.125).astype(np.float32)   # >= rowmax(s)
            qhh = np.concatenate([q_hi.T, q_hi.T]).astype(np.float32)
            qlo1 = np.concatenate([q_lo.T, -bias[None, :]]).astype(np.float32)
            in_maps.append({
                "khl": khl, "khi1": khi1, "qhh": qhh, "qlo1": qlo1,
                "vpk": vpk, "ident": ident,
            })
    return in_maps


def run(x, trace=False):
    from concourse.bass_utils import run_bass_kernel_spmd
    if "nc" not in _CACHE:
        _CACHE["nc"] = _build_nc()
    nc = _CACHE["nc"]
    in_maps = _prep_inputs(x)
    res = run_bass_kernel_spmd(
        nc, in_maps, core_ids=list(range(NCORES)), trace=trace,
    )
    outs = [res.results[i]["out"] for i in range(NCORES)]
    full = np.empty((B_, NTOK, C_), np.float32)
    for b in range(B_):
        full[b, :NQ] = outs[2 * b]
        full[b, NQ:] = outs[2 * b + 1]
    return full.reshape(B_, D_, H_, W_, C_), res


def kernel(x):
    out, _ = run(x, trace=False)
    return out
